# revision 18
# baseline (speedup 1.0000x reference)
"""Trainium2 Bass kernel for CorrLayerDownsample.

Math (reference): hatx = fft2(xpsi); per-moment p: corr = ifft2(h1 * conj(h2)).real,
masked by masks_shift[shifted[p]], keep union_idx positions.

Device algorithm (per core):
  - 2D DFT of the needed maps by PE matmuls, keeping only u = 0..64 rows
    (real-input Hermitian symmetry; row u>=65 of the spectrum is recovered in
    stage-2 via a x2 fold), laid out as hatxT[v=128 part, u=65 free].
  - Per moment, four elementwise products t_rr/t_ii/t_ir/t_ri (DVE+GPSIMD wide
    ops over runs of moments sharing the m1 map), which feed PE directly as
    stationary operands; the complex combine happens inside the PSUM
    accumulation group (no separate combine pass):
      T[u, {re|im}] = sum_v P[v,u] * Wn[v, {yd}]       (stage 1, 4 matmuls)
      out[yd, xd]   = sum_u T_re[u,yd] Wm_re[u,xd] - T_im[u,yd] Wm_im[u,xd]
  - Mask multiply + DMA out.

Sharding: 8 cores = batch b (4) x moment parity (2). The moment pair pattern is
identical for the two parities (a2 is the innermost index-generation loop), so
a single SPMD program works: per-core inputs carry b's maps and the parity's
m2-side map subset.
"""

import sys

sys.path.insert(0, "/opt/trn_rl_repo")

import numpy as np

J, B, C, M, N = 4, 4, 8, 128, 128
UH = M // 2 + 1  # 65 kept u rows
NCORES = 8

_CACHE = {}


def _host_prep(la1, la2, shifted, union_idx, masks_shift):
    """Index analysis. Returns None if the fast-path assumptions fail."""
    P = la1.shape[0]
    if P % 2 != 0:
        return None
    m1 = la1[:, 0].astype(np.int64) * C + la1[:, 1]
    m2 = la2[:, 0].astype(np.int64) * C + la2[:, 1]
    if (m1 < 0).any() or (m1 >= J * C).any() or (m2 < 0).any() or (m2 >= J * C).any():
        return None
    xs, ys = union_idx // N, union_idx % N
    X, Y = np.unique(xs), np.unique(ys)
    NX, NY = len(X), len(Y)
    if NX * NY != len(union_idx) or NX > 64 or NY > 64:
        return None
    gx, gy = np.meshgrid(X, Y, indexing="ij")
    if not np.array_equal(np.sort(union_idx), np.sort((gx * N + gy).ravel())):
        return None
    # union_idx must be sorted x-major for the final scatter to be a transpose
    if not np.array_equal(union_idx, (gx * N + gy).ravel()):
        return None
    pe, po = np.arange(0, P, 2), np.arange(1, P, 2)
    if not np.array_equal(m1[pe], m1[po]):
        return None
    sub_e, sub_o = np.unique(m2[pe]), np.unique(m2[po])
    if len(sub_e) > 16 or len(sub_o) > 16 or len(sub_e) != len(sub_o):
        return None
    slot_e = np.searchsorted(sub_e, m2[pe])
    slot_o = np.searchsorted(sub_o, m2[po])
    if not np.array_equal(slot_e, slot_o):
        return None
    if not np.array_equal(shifted[pe], shifted[po]):
        return None
    order = np.lexsort((slot_e, m1[pe]))  # sorted row order, same for both halves
    m1_s, slot_s = m1[pe][order], slot_e[order]
    runs = []  # (m1, slot0, count)
    i = 0
    while i < len(m1_s):
        j = i
        while (
            j < len(m1_s)
            and m1_s[j] == m1_s[i]
            and slot_s[j] == slot_s[i] + (j - i)
        ):
            j += 1
        runs.append((int(m1_s[i]), int(slot_s[i]), j - i))
        i = j
    if len(runs) > 64:
        return None
    return dict(
        m1=m1, m2=m2, X=X, Y=Y, NX=NX, NY=NY, pe=pe, po=po,
        sub_e=sub_e, sub_o=sub_o, order=order, runs=runs,
        n_rows=len(order), nsub=len(sub_e),
    )


def _consts(prep):
    X, Y, NX, NY = prep["X"], prep["Y"], prep["NX"], prep["NY"]
    k = np.arange(M)
    th = 2 * np.pi * np.outer(k, k[:UH]) / M
    FmRe = np.cos(th).astype(np.float32)          # [m, k1] lhsT of T1
    FmIm = (-np.sin(th)).astype(np.float32)
    thn = 2 * np.pi * np.outer(k, k) / N
    FnRe = np.cos(thn).astype(np.float32)         # [n, k2] lhsT of T2
    FnIm = (-np.sin(thn)).astype(np.float32)
    thw = 2 * np.pi * np.outer(k, Y) / N
    WnRe = (np.cos(thw) / N).astype(np.float32)   # [128, NY]
    WnIm = (np.sin(thw) / N).astype(np.float32)
    cu = np.full(UH, 2.0, np.float32)
    cu[0] = 1.0
    if M % 2 == 0:
        cu[UH - 1] = 1.0
    thm = 2 * np.pi * np.outer(np.arange(UH), X) / M
    WmRe = (cu[:, None] * np.cos(thm) / M).astype(np.float32)      # [65, NX]
    WmImNeg = (-cu[:, None] * np.sin(thm) / M).astype(np.float32)  # [65, NX]
    # Karatsuba 3-mult complex product: with m1=h1r*h2r, m2=h1i*h2i,
    # m3=(h1r+h1i)*(h2r-h2i):  P_re = m1+m2, P_im = m3-m1+m2.
    # T = P_re^T A + P_im^T B  =  m1^T(A-B) + m2^T(A+B) + m3^T B,
    # where A = [WnRe|WnIm], B = [-WnIm|WnRe].
    WnS1 = np.concatenate([WnRe + WnIm, WnIm - WnRe], axis=1)   # A - B
    WnS2 = np.concatenate([WnRe - WnIm, WnIm + WnRe], axis=1)   # A + B
    WnS3 = np.concatenate([-WnIm, WnRe], axis=1)                # B
    ident = np.eye(M, dtype=np.float32)
    return dict(
        FmRe=FmRe, FmIm=FmIm, FnRe=FnRe, FnIm=FnIm, FnImNeg=(-FnIm).copy(),
        WnS1=WnS1, WnS2=WnS2, WnS3=WnS3, WmRe=WmRe, WmImNeg=WmImNeg, ident=ident,
    )


def _build_program(prep, repeat=1):
    import concourse.bacc as bacc
    import concourse.mybir as mybir
    import concourse.tile as tile

    f32 = mybir.dt.float32
    NX, NY = prep["NX"], prep["NY"]
    n_rows, nsub = prep["n_rows"], prep["nsub"]
    runs = prep["runs"]
    nmaps = J * C + nsub  # 32 m1-side + nsub m2-side maps
    W2 = 2 * NY           # stacked stage-1 rhs width

    nc = bacc.Bacc("TRN2", target_bir_lowering=False, debug=False,
                   num_devices=NCORES)

    def din(name, shape):
        return nc.dram_tensor(name, list(shape), f32, kind="ExternalInput").ap()

    xmaps = din("xmaps", (nmaps, M, N))
    FmRe, FmIm = din("FmRe", (M, UH)), din("FmIm", (M, UH))
    FnRe, FnIm = din("FnRe", (M, M)), din("FnIm", (M, M))
    FnImNeg = din("FnImNeg", (M, M))
    WnS1, WnS2, WnS3 = din("WnS1", (M, W2)), din("WnS2", (M, W2)), din("WnS3", (M, W2))
    WmRe, WmImNeg = din("WmRe", (UH, NX)), din("WmImNeg", (UH, NX))
    ident = din("ident", (M, M))
    maskv = din("maskv", (NY, n_rows * NX))
    out = nc.dram_tensor("out", [n_rows, NY, NX], f32, kind="ExternalOutput").ap()

    GT = 512 // W2            # stage-1 T tiles per PSUM bank (15 for NY=17)
    GO = 512 // NX            # stage-2 outs per PSUM bank  (30 for NX=17)
    GO = min(GO, 16)

    with tile.TileContext(nc) as tc:
        with tc.tile_pool(name="const", bufs=1) as cpool:
            c_FmRe = cpool.tile([M, UH], f32)
            c_FmIm = cpool.tile([M, UH], f32)
            c_FnRe = cpool.tile([M, M], f32)
            c_FnIm = cpool.tile([M, M], f32)
            c_FnImNeg = cpool.tile([M, M], f32)
            c_Wn1 = cpool.tile([M, W2], f32)
            c_Wn2 = cpool.tile([M, W2], f32)
            c_Wn3 = cpool.tile([M, W2], f32)
            c_WmRe = cpool.tile([UH, NX], f32)
            c_WmImNeg = cpool.tile([UH, NX], f32)
            c_id = cpool.tile([M, M], f32)
            c_mask = cpool.tile([NY, n_rows * NX], f32)
            for t, s in [
                (c_FmRe, FmRe), (c_FmIm, FmIm), (c_FnRe, FnRe), (c_FnIm, FnIm),
                (c_FnImNeg, FnImNeg), (c_Wn1, WnS1), (c_Wn2, WnS2), (c_Wn3, WnS3),
                (c_WmRe, WmRe), (c_WmImNeg, WmImNeg), (c_id, ident), (c_mask, maskv),
            ]:
                nc.sync.dma_start(t[:], s[:])

            hat_ctx = tc.tile_pool(name="hatx", bufs=1)
            hat_pool = hat_ctx.__enter__()
            hat_re = hat_pool.tile([M, nmaps * UH], f32)
            hat_im = hat_pool.tile([M, nmaps * UH], f32)

            # ---------------- FFT phase ----------------
            # m2-side maps first so the main loop's first runs unblock early;
            # copies ride GPSIMD (DVE is idle here, ACT stays free for later).
            fft_order = list(range(J * C, nmaps)) + list(range(J * C))
            with tc.tile_pool(name="fftsb", bufs=4) as fsb, \
                 tc.tile_pool(name="fftps", bufs=2, space="PSUM") as fps, \
                 tc.tile_pool(name="fftps2", bufs=2, space="PSUM") as fps2, \
                 tc.tile_pool(name="fftps3", bufs=2, space="PSUM") as fps3:
                xbig = fsb.tile([M, nmaps * N], f32, tag="xbig")
                nc.sync.dma_start(
                    xbig[:].rearrange("p (z n) -> p z n", z=nmaps),
                    xmaps[:].transpose([1, 0, 2]))
                for _rep in range(repeat):
                  for z in fft_order:
                    xt = xbig[:, z * N:(z + 1) * N]
                    # T1: A[k1, n] = Fm^T x   (complex planes side by side)
                    pA = fps.tile([UH, 2 * N], f32, tag="pA")
                    nc.tensor.matmul(pA[:, 0:N], c_FmRe[:], xt, start=True, stop=True)
                    nc.tensor.matmul(pA[:, N:2 * N], c_FmIm[:], xt, start=True, stop=True)
                    sA = fsb.tile([UH, 2 * N], f32, tag="sA")
                    nc.vector.tensor_copy(sA[:], pA[:])
                    # transpose both planes -> AT [n, k1]
                    pT = fps2.tile([M, 2 * UH], f32, tag="pT")
                    nc.tensor.transpose(pT[:, 0:UH], sA[:, 0:N], c_id[0:UH, 0:UH])
                    nc.tensor.transpose(pT[:, UH:2 * UH], sA[:, N:2 * N], c_id[0:UH, 0:UH])
                    sT = fsb.tile([M, 2 * UH], f32, tag="sT")
                    nc.vector.tensor_copy(sT[:], pT[:])
                    # T2: B[k2, k1] = Fn^T AT (complex)
                    pB = fps3.tile([M, 2 * UH], f32, tag="pB")
                    nc.tensor.matmul(pB[:, 0:UH], c_FnRe[:], sT[:, 0:UH], start=True, stop=False)
                    nc.tensor.matmul(pB[:, 0:UH], c_FnImNeg[:], sT[:, UH:2 * UH], start=False, stop=True)
                    nc.tensor.matmul(pB[:, UH:2 * UH], c_FnRe[:], sT[:, UH:2 * UH], start=True, stop=False)
                    nc.tensor.matmul(pB[:, UH:2 * UH], c_FnIm[:], sT[:, 0:UH], start=False, stop=True)
                    nc.vector.tensor_copy(hat_re[:, z * UH:(z + 1) * UH], pB[:, 0:UH])
                    nc.vector.tensor_copy(hat_im[:, z * UH:(z + 1) * UH], pB[:, UH:2 * UH])

            # ---------------- main loop ----------------
            h2_re = hat_re[:, J * C * UH:]  # m2-side maps
            h2_im = hat_im[:, J * C * UH:]
            # Karatsuba sum planes: hs1 = h1r+h1i (m1 side), hs2 = h2r-h2i
            hs1 = hat_pool.tile([M, J * C * UH], f32)
            hs2 = hat_pool.tile([M, nsub * UH], f32)
            nc.vector.tensor_add(hs1[:], hat_re[:, :J * C * UH], hat_im[:, :J * C * UH])
            nc.vector.tensor_sub(hs2[:], h2_re, h2_im)

            with tc.tile_pool(name="tt", bufs=4) as tpool, \
                 tc.tile_pool(name="tsb", bufs=2) as tsbp, \
                 tc.tile_pool(name="stg", bufs=2) as stgp, \
                 tc.tile_pool(name="psT", bufs=3, space="PSUM") as psT, \
                 tc.tile_pool(name="psO", bufs=2, space="PSUM") as psO:

                # flat list of per-row t-tile APs, filled run by run
                row_t = [None] * (n_rows * repeat)
                r0 = 0
                for (a, s0, R) in runs * repeat:
                    t_m1 = tpool.tile([M, R * UH], f32, tag="t_m1")
                    t_m2 = tpool.tile([M, R * UH], f32, tag="t_m2")
                    t_m3 = tpool.tile([M, R * UH], f32, tag="t_m3")
                    a_re = hat_re[:, a * UH:(a + 1) * UH].unsqueeze(1).broadcast_to([M, R, UH])
                    a_im = hat_im[:, a * UH:(a + 1) * UH].unsqueeze(1).broadcast_to([M, R, UH])
                    a_s = hs1[:, a * UH:(a + 1) * UH].unsqueeze(1).broadcast_to([M, R, UH])
                    b_re = h2_re[:, s0 * UH:(s0 + R) * UH].rearrange("p (r u) -> p r u", r=R)
                    b_im = h2_im[:, s0 * UH:(s0 + R) * UH].rearrange("p (r u) -> p r u", r=R)
                    b_s = hs2[:, s0 * UH:(s0 + R) * UH].rearrange("p (r u) -> p r u", r=R)
                    v_m1 = t_m1[:].rearrange("p (r u) -> p r u", r=R)
                    v_m2 = t_m2[:].rearrange("p (r u) -> p r u", r=R)
                    v_m3 = t_m3[:].rearrange("p (r u) -> p r u", r=R)
                    nc.vector.tensor_mul(v_m1, a_re, b_re)
                    nc.vector.tensor_mul(v_m2, a_im, b_im)
                    nc.vector.tensor_mul(v_m3, a_s, b_s)
                    for i in range(R):
                        row_t[r0 + i] = (
                            t_m1[:, i * UH:(i + 1) * UH],
                            t_m2[:, i * UH:(i + 1) * UH],
                            t_m3[:, i * UH:(i + 1) * UH],
                        )
                    r0 += R

                # stage-1 groups of GT rows -> one PSUM bank + one batched copy
                Tsb = [None] * (n_rows * repeat)
                for g0 in range(0, n_rows * repeat, GT):
                    g = min(GT, n_rows * repeat - g0)
                    pT1 = psT.tile([UH, g * W2], f32, tag="pT1")
                    for i in range(g):
                        tm1, tm2, tm3 = row_t[g0 + i]
                        o = pT1[:, i * W2:(i + 1) * W2]
                        nc.tensor.matmul(o, tm1, c_Wn1[:], start=True, stop=False)
                        nc.tensor.matmul(o, tm2, c_Wn2[:], start=False, stop=False)
                        nc.tensor.matmul(o, tm3, c_Wn3[:], start=False, stop=True)
                    sT1 = tsbp.tile([UH, g * W2], f32, tag="sT1")
                    nc.scalar.copy(sT1[:], pT1[:])
                    for i in range(g):
                        Tsb[g0 + i] = sT1[:, i * W2:(i + 1) * W2]

                # stage-2 + mask + out DMA, groups of GO rows
                for g0 in range(0, n_rows * repeat, GO):
                    g = min(GO, n_rows * repeat - g0)
                    g0m = g0 % n_rows
                    pO = psO.tile([NY, g * NX], f32, tag="pO")
                    for i in range(g):
                        T = Tsb[g0 + i]
                        o = pO[:, i * NX:(i + 1) * NX]
                        nc.tensor.matmul(o, T[:, 0:NY], c_WmRe[:], start=True, stop=False)
                        nc.tensor.matmul(o, T[:, NY:2 * NY], c_WmImNeg[:], start=False, stop=True)
                    stg = stgp.tile([NY, g * NX], f32, tag="stg")
                    nc.vector.tensor_mul(stg[:], pO[:], c_mask[:, g0m * NX:(g0m + g) * NX])
                    nc.sync.dma_start(
                        out[g0m:g0m + g].transpose([1, 0, 2]),
                        stg[:].rearrange("p (r x) -> p r x", r=g),
                    )
            hat_ctx.__exit__(None, None, None)

    nc.compile()
    return nc


def _fallback(xpsi, masks_shift, la1, la2, shifted, union_idx):
    hatx = np.fft.fft2(xpsi.astype(np.float64))
    h1 = hatx[la1[:, 0], :, la1[:, 1]]
    h2 = hatx[la2[:, 0], :, la2[:, 1]]
    corr = np.fft.ifft2(h1 * np.conj(h2)).real
    masked = corr * masks_shift[shifted][:, None]
    Pm, Bb, Mm, Nn = masked.shape
    return masked.reshape(Pm, Bb, Mm * Nn)[:, :, union_idx].astype(np.float32)


def kernel(**inputs):
    xpsi = np.ascontiguousarray(np.asarray(inputs["xpsi"], dtype=np.float32))
    masks_shift = np.asarray(inputs["masks_shift"], dtype=np.float32)
    la1 = np.asarray(inputs["la1"], dtype=np.int64)
    la2 = np.asarray(inputs["la2"], dtype=np.int64)
    shifted = np.asarray(inputs["shifted"], dtype=np.int64)
    union_idx = np.asarray(inputs["union_idx"], dtype=np.int64)

    if xpsi.shape != (J, B, C, M, N) or (shifted < 0).any() or \
            (shifted >= masks_shift.shape[0]).any():
        return _fallback(xpsi, masks_shift, la1, la2, shifted, union_idx)
    prep = _host_prep(la1, la2, shifted, union_idx, masks_shift)
    if prep is None:
        return _fallback(xpsi, masks_shift, la1, la2, shifted, union_idx)

    key = (prep["NX"], prep["NY"], prep["n_rows"], tuple(prep["runs"]))
    if key not in _CACHE:
        _CACHE[key] = _build_program(prep)
    nc = _CACHE[key]

    cst = _consts(prep)
    X, Y, NX, NY = prep["X"], prep["Y"], prep["NX"], prep["NY"]
    pe, order = prep["pe"], prep["order"]
    n_rows = prep["n_rows"]

    # per-row mask values: maskv[y, row*NX + x] = masks[shifted[p], X[x], Y[y]]
    p_sorted_even = pe[order]                       # original even p per row
    mk = masks_shift[shifted[p_sorted_even]]        # [n_rows, 128, 128]
    mv = mk[:, X[:, None], Y[None, :]]              # [n_rows, NX, NY]
    maskv = np.ascontiguousarray(
        mv.transpose(2, 0, 1).reshape(NY, n_rows * NX))

    xflat = xpsi.transpose(0, 2, 1, 3, 4).reshape(J * C, B, M, N)
    in_maps = []
    for core in range(NCORES):
        b, parity = divmod(core, 2)
        sub = prep["sub_e"] if parity == 0 else prep["sub_o"]
        xm = np.concatenate(
            [xflat[:, b], xflat[sub, b]], axis=0).astype(np.float32)
        in_maps.append({
            "xmaps": np.ascontiguousarray(xm),
            "FmRe": cst["FmRe"], "FmIm": cst["FmIm"],
            "FnRe": cst["FnRe"], "FnIm": cst["FnIm"], "FnImNeg": cst["FnImNeg"],
            "WnS1": cst["WnS1"], "WnS2": cst["WnS2"], "WnS3": cst["WnS3"],
            "WmRe": cst["WmRe"], "WmImNeg": cst["WmImNeg"],
            "ident": cst["ident"], "maskv": maskv,
        })

    from concourse.bass_utils import run_bass_kernel_spmd
    res = run_bass_kernel_spmd(nc, in_maps, list(range(NCORES)))

    P = la1.shape[0]
    out = np.empty((P, B, len(union_idx)), np.float32)
    inv = np.empty(n_rows, np.int64)
    inv[order] = np.arange(n_rows)                  # row of sorted order for pe[k]
    for core in range(NCORES):
        b, parity = divmod(core, 2)
        dev = res.results[core]["out"]              # [n_rows, NY, NX]
        flat = dev.transpose(0, 2, 1).reshape(n_rows, NX * NY)  # x-major
        p_idx = prep["pe"] if parity == 0 else prep["po"]
        out[p_idx, b, :] = flat[inv]
    return out


if __name__ == "__main__":
    import importlib
    ref = importlib.import_module("reference")
    import jax
    cpu = jax.devices("cpu")[0]
    with jax.default_device(cpu):
        raw = ref.setup_inputs()
        ins = {k: np.asarray(v) for k, v in raw.items()}
        exp = np.asarray(ref.reference(**{k: jax.device_put(v, cpu) for k, v in raw.items()}))
    got = kernel(**ins)
    d = np.linalg.norm(got - exp) / np.linalg.norm(exp)
    print("rel:", d, "maxabs:", np.abs(got - exp).max())
